# revision 1
# baseline (speedup 1.0000x reference)
"""Graph-Transformer (3-layer) Trainium2 kernel, 8-core SPMD.

Sharding: nodes (and edges by destination) split across 8 cores.
Per layer: per-core QKV for the local node slice, AllGather of packed
k|v tables (bf16), per-edge-block indirect-DMA gather of source k|v
rows, PE-matmul aggregation with host-built fp8 one-hot assignment
matrices, fused attention/FFN/LayerNorm x-update per 128-node tile.
Readout mean+MLP (10K FLOPs) done on host from per-core partial sums.
"""
import sys

sys.path.insert(0, "/opt/trn_rl_repo")

import contextlib
import numpy as np
import ml_dtypes

import concourse.bass as bass
import concourse.tile as tile
from concourse import bacc, mybir
from concourse.bass_utils import run_bass_kernel_spmd
from concourse.masks import make_identity

P = 128
NCORES = 8
N, E, D, H, L = 50000, 800000, 128, 8, 3
HD = D // H
VOCAB = 7000
NL = N // NCORES            # 6250 nodes per core
NT = (NL + P - 1) // P      # 49 windows/tiles per core
LAST = NL - (NT - 1) * P    # 106 rows in last tile
VT = (VOCAB + P - 1) // P   # 55 vocab tiles
VLAST = VOCAB - (VT - 1) * P
F32 = mybir.dt.float32
BF16 = mybir.dt.bfloat16
FP8 = mybir.dt.float8e4
I32 = mybir.dt.int32
ADD = mybir.AluOpType.add
MUL = mybir.AluOpType.mult
SUB = mybir.AluOpType.subtract
AF = mybir.ActivationFunctionType

_CACHE = {}
NO_COLLECTIVE = False


def _preprocess_edges(edge_index):
    e0 = np.asarray(edge_index[0], dtype=np.int64)
    e1 = np.asarray(edge_index[1], dtype=np.int64)
    core = e1 // NL
    dl = e1 - core * NL
    w = dl >> 7
    cw = core * NT + w
    cnt = np.bincount(cw, minlength=NCORES * NT).reshape(NCORES, NT)
    K_w = np.maximum(1, -(-cnt.max(axis=0) // P))
    NBLK = int(K_w.sum())
    blk_start = np.concatenate([[0], np.cumsum(K_w)])[:-1]

    src = np.zeros((NCORES, NBLK * P), np.int32)
    dwin = np.full((NCORES, NBLK * P), 255, np.int16)
    for c in range(NCORES):
        m = core == c
        ec0, ew, edl = e0[m], w[m], dl[m]
        order = np.argsort(ew, kind="stable")
        ec0, ew, edl = ec0[order], ew[order], edl[order]
        cnts = cnt[c]
        run_start = np.concatenate([[0], np.cumsum(cnts)])[:-1]
        pos = np.arange(len(ec0)) - np.repeat(run_start, cnts)
        slot = np.repeat(blk_start * P, cnts) + pos
        src[c, slot] = ec0
        dwin[c, slot] = (edl & 127).astype(np.int16)

    src = src.reshape(NCORES, NBLK, P)
    dwin = dwin.reshape(NCORES, NBLK, P)
    eye = np.arange(P, dtype=np.int16)
    A = dwin[:, :, :, None] == eye[None, None, None, :]
    A8 = A.astype(ml_dtypes.float8_e4m3fn)
    AT8 = np.ascontiguousarray(A8.transpose(0, 1, 3, 2))
    srcT = np.ascontiguousarray(src.transpose(0, 2, 1)).astype(np.int32)
    return K_w, NBLK, srcT, A8, AT8


def _build_program(K_w, NBLK, has_bias, has_ln_aff):
    nc = bacc.Bacc("TRN2", target_bir_lowering=False, debug=False,
                   num_devices=NCORES)
    dt = nc.dram_tensor
    tn = {}
    tn["emb"] = dt("emb", [VT * P, D], F32, kind="ExternalInput")
    tn["ilocT"] = dt("ilocT", [P, NT], I32, kind="ExternalInput")
    tn["svocT"] = dt("svocT", [P, NBLK], I32, kind="ExternalInput")
    tn["snodT"] = dt("snodT", [P, NBLK], I32, kind="ExternalInput")
    tn["Ablob"] = dt("Ablob", [NBLK, P, P], FP8, kind="ExternalInput")
    tn["ATblob"] = dt("ATblob", [NBLK, P, P], FP8, kind="ExternalInput")
    for nm, sh in [("Wq", [L, D, D]), ("Wk", [L, D, D]), ("Wv", [L, D, D]),
                   ("Wo", [L, D, D]), ("Wf1", [L, D, 2 * D]),
                   ("Wf2", [L, 2 * D, D]), ("biasb", [L, 9, P, 2 * D])]:
        tn[nm] = dt(nm, sh, F32, kind="ExternalInput")
    tn["out"] = dt("out", [1, D], F32, kind="ExternalOutput")

    with tile.TileContext(nc) as tc:
        _emit(nc, tc, tn, K_w, NBLK, has_bias, has_ln_aff)
    nc.compile()
    return nc


def _emit(nc, tc, tn, K_w, NBLK, has_bias, has_ln_aff):
    blk_start = np.concatenate([[0], np.cumsum(K_w)])[:-1]
    ctx = contextlib.ExitStack()
    with ctx:
        stat = ctx.enter_context(tc.tile_pool(name="stat", bufs=1))
        sb = ctx.enter_context(tc.tile_pool(name="sb", bufs=3))
        awin = ctx.enter_context(tc.tile_pool(name="awin", bufs=2))
        kvp = ctx.enter_context(tc.tile_pool(name="kvp", bufs=4))
        pst = ctx.enter_context(tc.tile_pool(name="pst", bufs=2, space="PSUM"))
        psq = ctx.enter_context(tc.tile_pool(name="psq", bufs=2, space="PSUM"))
        psw = ctx.enter_context(tc.tile_pool(name="psw", bufs=2, space="PSUM"))
        psl = ctx.enter_context(tc.tile_pool(name="psl", bufs=2, space="PSUM"))
        dram = ctx.enter_context(tc.tile_pool(name="dram", bufs=1, space="DRAM"))
        dram2 = ctx.enter_context(tc.tile_pool(name="dram2", bufs=2, space="DRAM"))

        ident = stat.tile([P, P], F32)
        make_identity(nc, ident[:])
        ones_col = stat.tile([P, 1], F32)
        nc.gpsimd.memset(ones_col[:], 1.0)
        epsln = stat.tile([P, 1], F32)
        nc.gpsimd.memset(epsln[:], 1e-5)

        wts = {}
        for nm in ["Wq", "Wk", "Wv", "Wo", "Wf1"]:
            for l in range(L):
                t = stat.tile(list(tn[nm].shape[1:]), F32, tag=f"{nm}{l}",
                              name=f"{nm}{l}")
                nc.sync.dma_start(t[:], tn[nm][l])
                wts[(nm, l)] = t
        for l in range(L):
            for half in range(2):
                t = stat.tile([D, D], F32, tag=f"Wf2{l}h{half}",
                              name=f"Wf2{l}h{half}")
                nc.sync.dma_start(t[:], tn["Wf2"][l, half * D: (half + 1) * D])
                wts[("Wf2", l, half)] = t
        bias = []
        for l in range(L):
            t = stat.tile([P, 9 * 2 * D], F32, tag=f"bias{l}", name=f"bias{l}")
            nc.sync.dma_start(t[:].rearrange("p (n d) -> p n d", n=9),
                              tn["biasb"][l].rearrange("n p d -> p n d"))
            bias.append(t)

        def bslice(l, i, width=D, off=0):
            return bias[l][:, i * 2 * D + off: i * 2 * D + off + width]

        ilocT = stat.tile([P, NT], I32)
        nc.sync.dma_start(ilocT[:], tn["ilocT"][:])
        svocT = stat.tile([P, NBLK], I32)
        nc.sync.dma_start(svocT[:], tn["svocT"][:])
        snodT = stat.tile([P, NBLK], I32)
        nc.sync.dma_start(snodT[:], tn["snodT"][:])

        x_sb = stat.tile([P, NT * D], F32)
        x_v = x_sb[:].rearrange("p (t d) -> p t d", t=NT)
        q_sb = stat.tile([P, NT * D], BF16)
        q_v = q_sb[:].rearrange("p (t d) -> p t d", t=NT)
        acc = stat.tile([P, D], F32)
        nc.gpsimd.memset(acc[:], 0.0)

        embkv = dram.tile([VT * P, 2 * D], BF16)
        embq = dram.tile([VT * P, D], BF16)
        kvloc = dram2.tile([NL, 2 * D], BF16, tag="kvloc")

        def linear(xT_sbuf, w_tile, width=D):
            o = psl.tile([P, 2 * D], F32, space="PSUM", tag="lin", name="lin")
            nc.tensor.matmul(o[:, :width], lhsT=xT_sbuf, rhs=w_tile,
                             start=True, stop=True)
            return o[:, :width]

        def transpose_sb(src_ap, name):
            tps = pst.tile([P, P], F32, space="PSUM", tag="tps", name="tps")
            nc.tensor.transpose(tps[:], in_=src_ap, identity=ident[:])
            t_sb = sb.tile([P, P], F32, tag="tsb", name=name)
            nc.vector.tensor_copy(t_sb[:], tps[:])
            return t_sb

        # ---- phase 0: embedding-projection tables for layer 1 ----
        for vt in range(VT):
            rows = VLAST if vt == VT - 1 else P
            e_tl = sb.tile([P, D], F32, tag="etile", name="etile")
            nc.sync.dma_start(e_tl[:rows], tn["emb"][vt * P: vt * P + rows])
            eT = transpose_sb(e_tl[:], "eT")
            q_ps = linear(eT[:], wts[("Wq", 0)][:])
            k_ps = linear(eT[:], wts[("Wk", 0)][:])
            v_ps = linear(eT[:], wts[("Wv", 0)][:])
            kv_o = sb.tile([P, 2 * D], BF16, tag="kvo", name="kvo")
            q_o = sb.tile([P, D], BF16, tag="qo", name="qo")
            if has_bias[0]:
                nc.vector.tensor_tensor(out=q_o[:], in0=q_ps, in1=bslice(0, 0), op=ADD)
                nc.vector.tensor_tensor(out=kv_o[:, :D], in0=k_ps, in1=bslice(0, 1), op=ADD)
                nc.vector.tensor_tensor(out=kv_o[:, D:], in0=v_ps, in1=bslice(0, 2), op=ADD)
            else:
                nc.vector.tensor_copy(q_o[:], q_ps)
                nc.vector.tensor_copy(kv_o[:, :D], k_ps)
                nc.vector.tensor_copy(kv_o[:, D:], v_ps)
            nc.sync.dma_start(embq[vt * P: vt * P + P], q_o[:])
            nc.sync.dma_start(embkv[vt * P: vt * P + P], kv_o[:])

        # ---- phase 0b: gather local x0 (residual input) and q for layer 1
        for t in range(NT):
            nc.gpsimd.indirect_dma_start(
                out=x_v[:, t, :], out_offset=None, in_=tn["emb"][:],
                in_offset=bass.IndirectOffsetOnAxis(ap=ilocT[:, t: t + 1], axis=0))
            nc.gpsimd.indirect_dma_start(
                out=q_v[:, t, :], out_offset=None, in_=embq[:],
                in_offset=bass.IndirectOffsetOnAxis(ap=ilocT[:, t: t + 1], axis=0))

        def layer_norm(dst_ap, src_ap, res_ap, l, which):
            """dst = LN(src + res) * g + b, all [P, D] rows."""
            r = sb.tile([P, D], F32, tag="lnr", name="lnr")
            nc.vector.tensor_tensor(out=r[:], in0=src_ap, in1=res_ap, op=ADD)
            mcol = sb.tile([P, 1], F32, tag="lnm", name="lnm")
            nc.vector.tensor_reduce(out=mcol[:], in_=r[:],
                                    axis=mybir.AxisListType.X, op=ADD)
            nc.vector.tensor_scalar_mul(mcol[:], mcol[:], 1.0 / D)
            cen = sb.tile([P, D], F32, tag="lncen", name="lncen")
            nc.vector.tensor_tensor(out=cen[:], in0=r[:],
                                    in1=mcol[:].to_broadcast([P, D]), op=SUB)
            sq = sb.tile([P, D], F32, tag="lnsq", name="lnsq")
            vcol = sb.tile([P, 1], F32, tag="lnv", name="lnv")
            nc.scalar.activation(out=sq[:], in_=cen[:], func=AF.Square,
                                 accum_out=vcol[:])
            std = sb.tile([P, 1], F32, tag="lnstd", name="lnstd")
            nc.scalar.activation(out=std[:], in_=vcol[:], func=AF.Sqrt,
                                 bias=epsln[:], scale=1.0 / D)
            rstd = sb.tile([P, 1], F32, tag="lnrstd", name="lnrstd")
            nc.vector.reciprocal(rstd[:], std[:])
            if has_ln_aff[l]:
                goff = 0 if which == 1 else D
                tmp = sb.tile([P, D], F32, tag="lntmp", name="lntmp")
                nc.vector.tensor_tensor(out=tmp[:], in0=cen[:],
                                        in1=rstd[:].to_broadcast([P, D]), op=MUL)
                bi = 6 if which == 1 else 7
                nc.vector.scalar_tensor_tensor(
                    out=tmp[:], in0=tmp[:], scalar=1.0,
                    in1=bslice(l, 8, D, goff), op0=MUL, op1=MUL)
                nc.vector.tensor_tensor(out=dst_ap, in0=tmp[:],
                                        in1=bslice(l, bi), op=ADD)
            else:
                nc.vector.tensor_tensor(out=dst_ap, in0=cen[:],
                                        in1=rstd[:].to_broadcast([P, D]), op=MUL)

        kvfull_l = None
        for l in range(L):
            if l == 0:
                table, idxT = embkv, svocT
            else:
                table, idxT = kvfull_l, snodT
            for w in range(NT):
                K = int(K_w[w])
                bs = int(blk_start[w])
                a_w = awin.tile([P, K * P], FP8, tag="aw", name="aw")
                at_w = awin.tile([P, K * P], FP8, tag="atw", name="atw")
                nc.sync.dma_start(
                    a_w[:].rearrange("e (k n) -> e k n", k=K),
                    tn["Ablob"][bs: bs + K].rearrange("k e n -> e k n"))
                nc.sync.dma_start(
                    at_w[:].rearrange("n (k e) -> n k e", k=K),
                    tn["ATblob"][bs: bs + K].rearrange("k n e -> n k e"))
                a_v = a_w[:].rearrange("e (k n) -> e k n", k=K)
                at_v = at_w[:].rearrange("n (k e) -> n k e", k=K)
                pw = psw.tile([P, 136], F32, space="PSUM", tag="pw", name="pw")
                for j in range(K):
                    b = bs + j
                    kvb = kvp.tile([P, 2 * D], BF16, tag="kvb", name="kvb")
                    nc.gpsimd.indirect_dma_start(
                        out=kvb[:], out_offset=None, in_=table[:],
                        in_offset=bass.IndirectOffsetOnAxis(
                            ap=idxT[:, b: b + 1], axis=0))
                    qb = psq.tile([P, D], F32, space="PSUM", tag="qb", name="qb")
                    nc.tensor.matmul(qb[:], lhsT=at_v[:, j, :], rhs=q_v[:, w, :],
                                     start=True, stop=True)
                    tmul = sb.tile([P, D], BF16, tag="tmul", name="tmul")
                    nc.vector.tensor_tensor(out=tmul[:], in0=kvb[:, :D],
                                            in1=qb[:], op=MUL)
                    s_c = sb.tile([P, H], F32, tag="sc", name="sc")
                    nc.vector.tensor_reduce(
                        out=s_c[:], in_=tmul[:].rearrange("p (h c) -> p h c", h=H),
                        axis=mybir.AxisListType.X, op=ADD)
                    wexp = sb.tile([P, H], F32, tag="wexp", name="wexp")
                    nc.scalar.activation(out=wexp[:], in_=s_c[:], func=AF.Exp,
                                         scale=float(1.0 / np.sqrt(HD)))
                    rhs = sb.tile([P, 136], BF16, tag="rhs", name="rhs")
                    nc.vector.tensor_tensor(
                        out=rhs[:, :D].rearrange("p (h c) -> p h c", h=H),
                        in0=kvb[:, D:].rearrange("p (h c) -> p h c", h=H),
                        in1=wexp[:, :, None].to_broadcast([P, H, HD]), op=MUL)
                    nc.vector.tensor_copy(rhs[:, D:136], wexp[:])
                    nc.tensor.matmul(pw[:], lhsT=a_v[:, j, :], rhs=rhs[:],
                                     start=(j == 0), stop=(j == K - 1))
                zr = sb.tile([P, H], F32, tag="zr", name="zr")
                nc.vector.tensor_scalar_add(zr[:], pw[:, D:136], 1e-6)
                nc.vector.reciprocal(zr[:], zr[:])
                att = sb.tile([P, D], F32, tag="att", name="att")
                nc.vector.tensor_tensor(
                    out=att[:].rearrange("p (h c) -> p h c", h=H),
                    in0=pw[:, :D].rearrange("p (h c) -> p h c", h=H),
                    in1=zr[:, :, None].to_broadcast([P, H, HD]), op=MUL)
                # ---- x-update for tile w ----
                aT = transpose_sb(att[:], "attT")
                y_ps = linear(aT[:], wts[("Wo", l)][:])
                y = sb.tile([P, D], F32, tag="ytile", name="ytile")
                if has_bias[l]:
                    nc.vector.tensor_tensor(out=y[:], in0=y_ps, in1=bslice(l, 3), op=ADD)
                else:
                    nc.vector.tensor_copy(y[:], y_ps)
                x1 = sb.tile([P, D], F32, tag="x1", name="x1")
                layer_norm(x1[:], y[:], x_v[:, w, :], l, 1)
                yT = transpose_sb(y[:], "yT")
                f1_ps = linear(yT[:], wts[("Wf1", l)][:], width=2 * D)
                h1 = sb.tile([P, 2 * D], F32, tag="h1", name="h1")
                if has_bias[l]:
                    nc.vector.tensor_tensor(out=h1[:], in0=f1_ps,
                                            in1=bslice(l, 4, 2 * D), op=ADD)
                    nc.scalar.activation(out=h1[:], in_=h1[:], func=AF.Relu)
                else:
                    nc.scalar.activation(out=h1[:], in_=f1_ps, func=AF.Relu)
                h1Ta = transpose_sb(h1[:, :D], "h1Ta")
                h1Tb = transpose_sb(h1[:, D:], "h1Tb")
                f2_ps = psl.tile([P, 2 * D], F32, space="PSUM", tag="lin", name="f2")
                nc.tensor.matmul(f2_ps[:, :D], lhsT=h1Ta[:],
                                 rhs=wts[("Wf2", l, 0)][:], start=True, stop=False)
                nc.tensor.matmul(f2_ps[:, :D], lhsT=h1Tb[:],
                                 rhs=wts[("Wf2", l, 1)][:], start=False, stop=True)
                y2 = sb.tile([P, D], F32, tag="y2", name="y2")
                if has_bias[l]:
                    nc.vector.tensor_tensor(out=y2[:], in0=f2_ps[:, :D],
                                            in1=bslice(l, 5), op=ADD)
                else:
                    nc.vector.tensor_copy(y2[:], f2_ps[:, :D])
                layer_norm(x_v[:, w, :], y2[:], x1[:], l, 2)
                rows = LAST if w == NT - 1 else P
                if l < L - 1:
                    xT = transpose_sb(x_v[:, w, :], "xT")
                    nq = linear(xT[:], wts[("Wq", l + 1)][:])
                    nk = linear(xT[:], wts[("Wk", l + 1)][:])
                    nv = linear(xT[:], wts[("Wv", l + 1)][:])
                    kvo = sb.tile([P, 2 * D], BF16, tag="kvo", name="kvol")
                    if has_bias[l + 1]:
                        nc.vector.tensor_tensor(out=q_v[:, w, :], in0=nq,
                                                in1=bslice(l + 1, 0), op=ADD)
                        nc.vector.tensor_tensor(out=kvo[:, :D], in0=nk,
                                                in1=bslice(l + 1, 1), op=ADD)
                        nc.vector.tensor_tensor(out=kvo[:, D:], in0=nv,
                                                in1=bslice(l + 1, 2), op=ADD)
                    else:
                        nc.vector.tensor_copy(q_v[:, w, :], nq)
                        nc.vector.tensor_copy(kvo[:, :D], nk)
                        nc.vector.tensor_copy(kvo[:, D:], nv)
                    nc.sync.dma_start(kvloc[w * P: w * P + rows], kvo[:rows])
                else:
                    nc.vector.tensor_tensor(out=acc[:rows], in0=acc[:rows],
                                            in1=x_v[:rows, w, :], op=ADD)
            if l < L - 1:
                kvfull_l = dram2.tile([N, 2 * D], BF16, tag="kvfull", name="kvfull",
                                      addr_space="Shared")
                if NO_COLLECTIVE:
                    nc.sync.dma_start(kvfull_l[:NL], kvloc[:])
                else:
                    nc.gpsimd.collective_compute(
                        "AllGather", mybir.AluOpType.bypass,
                        replica_groups=[list(range(NCORES))],
                        ins=[kvloc[:].opt()], outs=[kvfull_l[:].opt()])

        o_ps = pst.tile([P, P], F32, space="PSUM", tag="tps", name="ops")
        nc.tensor.matmul(o_ps[:1, :D], lhsT=ones_col[:], rhs=acc[:],
                         start=True, stop=True)
        o_sb = sb.tile([1, D], F32, tag="osb", name="osb")
        nc.vector.tensor_copy(o_sb[:], o_ps[:1, :D])
        nc.sync.dma_start(tn["out"][:], o_sb[:])


def kernel(**inputs):
    x_idx = np.asarray(inputs["x_idx"]).reshape(N).astype(np.int64)
    edge_index = np.asarray(inputs["edge_index"])
    getf = lambda k: np.asarray(inputs[k], np.float32)
    emb = getf("emb")
    bq, bk, bv, bo = getf("bq"), getf("bk"), getf("bv"), getf("bo")
    bf1, bf2 = getf("bf1"), getf("bf2")
    g1, be1, g2, be2 = getf("g1"), getf("be1"), getf("g2"), getf("be2")

    K_w, NBLK, srcT, A8, AT8 = _preprocess_edges(edge_index)
    has_bias = [bool(np.any(bq[l]) or np.any(bk[l]) or np.any(bv[l])
                     or np.any(bo[l]) or np.any(bf1[l]) or np.any(bf2[l]))
                for l in range(L)]
    has_ln_aff = [bool(np.any(g1[l] != 1) or np.any(be1[l])
                       or np.any(g2[l] != 1) or np.any(be2[l]))
                  for l in range(L)]

    prog_key = (tuple(K_w.tolist()), tuple(has_bias), tuple(has_ln_aff))
    if prog_key not in _CACHE:
        _CACHE[prog_key] = _build_program(K_w, NBLK, has_bias, has_ln_aff)
    nc = _CACHE[prog_key]

    emb_pad = np.zeros((VT * P, D), np.float32)
    emb_pad[:VOCAB] = emb
    bias_blob = np.zeros((L, 9, P, 2 * D), np.float32)
    for l in range(L):
        for i, bvec in enumerate([bq[l], bk[l], bv[l], bo[l], bf1[l], bf2[l],
                                  be1[l], be2[l]]):
            bias_blob[l, i, :, : len(bvec)] = bvec[None, :]
        bias_blob[l, 8, :, :D] = g1[l][None, :]
        bias_blob[l, 8, :, D:] = g2[l][None, :]
    iloc_pad = np.zeros((NCORES, NT * P), np.int32)
    for c in range(NCORES):
        iloc_pad[c, :NL] = x_idx[c * NL: (c + 1) * NL]
    ilocT = iloc_pad.reshape(NCORES, NT, P).transpose(0, 2, 1)
    srcv = x_idx[srcT]

    shared = {"emb": emb_pad, "biasb": bias_blob}
    for nm in ["Wq", "Wk", "Wv", "Wo", "Wf1", "Wf2"]:
        shared[nm] = getf(nm)
    in_maps = []
    for c in range(NCORES):
        m = dict(shared)
        m["ilocT"] = np.ascontiguousarray(ilocT[c])
        m["svocT"] = np.ascontiguousarray(srcv[c]).astype(np.int32)
        m["snodT"] = np.ascontiguousarray(srcT[c])
        m["Ablob"] = A8[c]
        m["ATblob"] = AT8[c]
        in_maps.append(m)

    kernel.last_nc = nc
    kernel.last_in_maps = in_maps
    res = run_bass_kernel_spmd(nc, in_maps, list(range(NCORES)),
                               **getattr(kernel, "run_kwargs", {}))
    kernel.last_results = res
    total = np.zeros((1, D), np.float32)
    for c in range(NCORES):
        total += res.results[c]["out"]
    xm = total / N
    o = np.maximum(xm @ getf("mW0") + getf("mb0"), 0.0)
    o = np.maximum(o @ getf("mW1") + getf("mb1"), 0.0)
    return (o @ getf("mW2") + getf("mb2")).astype(np.float32)

